# revision 25
# baseline (speedup 1.0000x reference)
"""EnhancedGDN Trainium2 kernel.

Data-parallel over batch B=64 across 8 NeuronCores (8 graphs each).
All 64 graphs share one edge list -> host does integer-only index prep
(dedup + CSR padding); all float math runs on device.

Math (per device, 8 graphs):
  t_out = data @ v_w.T + v_b                  (temporal attn: window=1 -> softmax==1)
  x     = data @ lin_w.T
  s_i/s_j per node from att vectors (+ tiled emb scores)
  per-edge: alpha = lrelu(s_i[dst] + s_j[src]); softmax over in-edges of dst
  agg   = sum_e w_e * x[src]  (dense per-graph W matmul; W built by local_scatter)
  BatchNorm over all 64k nodes (AllReduce of sums) + ReLU
  out   = (relu([s_out|t_out] @ f_w1.T + f_b1) @ f_w2.T + f_b2) @ out_w.T + out_b
"""

import os

os.environ.setdefault("NEURON_RT_RESET_CORES", "1")

import numpy as np

import concourse.bass as bass
import concourse.bacc as bacc
import concourse.tile as tile
from concourse import mybir
from concourse.bass_utils import run_bass_kernel_spmd

B, N, D, E = 64, 1000, 128, 20000
M = 8          # devices
G = B // M     # graphs per device
NG = G * N     # nodes per device
NEG = 0.2
EPS = 1e-5

F16 = mybir.dt.float16
F32 = mybir.dt.float32
I16 = mybir.dt.int16
AF = mybir.ActivationFunctionType
ALU = mybir.AluOpType

_CACHE = {}


# ---------------------------------------------------------------- host index prep
def _prep_indices(edge_index):
    src = edge_index[0].astype(np.int64)
    dst = edge_index[1].astype(np.int64)
    key = dst * N + src
    uniq, cnt = np.unique(key, return_counts=True)
    ii = uniq // N
    jj = uniq % N
    # add self loops (reference removes none exist, then adds them)
    ii = np.concatenate([ii, np.arange(N)])
    jj = np.concatenate([jj, np.arange(N)])
    cc = np.concatenate([cnt, np.ones(N, np.int64)]).astype(np.float32)

    def _round4(k):
        return max(4, (k + 3) // 4 * 4)


    # out-CSR (grouped by src j)
    order = np.argsort(jj, kind="stable")
    oj, oi, oc = jj[order], ii[order], cc[order]
    outdeg = np.bincount(oj, minlength=N)
    Kout = _round4(int(outdeg.max()))
    out_i = np.full((1024, Kout), 1000, np.int64)  # gather pads -> sentinel
    out_i_sc = np.full((1024, Kout), -1, np.int64)  # -1 pads for local_scatter
    out_c = np.zeros((1024, Kout), np.float32)
    starts = np.searchsorted(oj, np.arange(N))
    kpos = np.arange(len(oj)) - starts[oj]
    out_i[oj, kpos] = oi
    out_i_sc[oj, kpos] = oi
    out_c[oj, kpos] = oc

    def _wrap(flat_per_core):
        # flat_per_core: [8, 128*K] -> wrapped [128, 128*K//16]
        nidx = flat_per_core.shape[1]
        w = np.zeros((128, nidx // 16), np.int16)
        for c in range(8):
            for p in range(16):
                w[16 * c + p] = flat_per_core[c, p::16]
        return w

    out_flat = out_i.reshape(8, 128 * Kout)       # core c -> src chunk c
    outidx = _wrap(out_flat)
    outcnt = np.repeat(out_c.reshape(8, 128 * Kout), 16, axis=0).astype(np.float16)
    scatidx = out_i_sc.reshape(8, 128, Kout).transpose(1, 0, 2).reshape(128, 8 * Kout)
    scatidx = np.ascontiguousarray(scatidx).astype(np.int16)
    return dict(Kout=Kout, outidx=outidx, outcnt=outcnt, scatidx=scatidx)


# ---------------------------------------------------------------- device module
def _build(Kout, n_cores, dbg=False):
    NOUT = 128 * Kout   # per-core src-phase slots
    NH = 16             # eviction chunks of 500 over 8000
    CH = 500

    nc = bacc.Bacc("TRN2", target_bir_lowering=False, debug=False,
                   num_devices=n_cores)

    def din(name, shape, dt):
        return nc.dram_tensor(name, shape, dt, kind="ExternalInput").ap()

    x0T = din("x0T", [128, NG], F16)
    embT = din("embT", [128, N], F16)
    wpack = din("wpack", [128, 1056], F16)
    bpack = din("bpack", [128, 8], F32)
    outb = din("outb", [1, 1], F32)
    outidx_d = din("outidx", [128, NOUT // 16], I16)
    outcnt_d = din("outcnt", [128, NOUT], F16)
    scatidx_d = din("scatidx", [128, 8 * Kout], I16)
    y_out = nc.dram_tensor("y", [1, NG], F32, kind="ExternalOutput").ap()

    if dbg:
        dbg_outs = {}
        def dout(name, shape, dt):
            ap = nc.dram_tensor(name, shape, dt, kind="ExternalOutput").ap()
            dbg_outs[name] = ap
            return ap
        d_siN = dout("d_siN", [8, 1024], F32)
        d_sjN = dout("d_sjN", [8, 1024], F32)
        d_cw = dout("d_cw", [128, 128 * Kout], F16)
        d_agg = dout("d_agg", [128, NG], F16)
        d_stats = dout("d_stats", [128, 2], F32)
        d_gstats = dout("d_gstats", [128, 2], F32)
        d_cf = dout("d_cf", [128, 6], F32)
        d_xT = dout("d_xT", [128, NG], F16)
        d_tT = dout("d_tT", [128, NG], F16)
        d_hT = dout("d_hT", [128, NG], F16)
        d_wt = dout("d_wt", [128, 8000], F16)

    cc_in = nc.dram_tensor("cc_in", [128, 2], F32).ap()
    cc_out = nc.dram_tensor("cc_out", [128, 2], F32, addr_space="Shared").ap()
    cc_win = nc.dram_tensor("cc_win", [128, 2], F32).ap()
    cc_wout = nc.dram_tensor("cc_wout", [128, 2], F32, addr_space="Shared").ap()

    # wpack column layout
    W_LIN, W_V, W_F2, W_F1, W_ATTQ, W_ATTEM, W_OUT, W_ID, W_ONE = (
        0, 128, 256, 384, 640, 642, 644, 645, 773)
    W_ONER = 774
    W_F2P = 902
    # bpack columns: v_b, gnn_bias, f_b1, f_b2, bn_gamma, bn_beta
    B_VB, B_GNN, B_FB1, B_FB2, B_GAM, B_BET, B_EPS = 0, 1, 2, 3, 4, 5, 6

    with tile.TileContext(nc) as tc:
        with (
            tc.tile_pool(name="cst", bufs=1) as cst,
            tc.tile_pool(name="big", bufs=1) as big,
            tc.tile_pool(name="wt", bufs=3) as wtp,
            tc.tile_pool(name="sm", bufs=1) as sm,
            tc.tile_pool(name="stg", bufs=4) as stg,
            tc.tile_pool(name="wd", bufs=2) as wdp,
            tc.tile_pool(name="dnp", bufs=2) as dnp,
            tc.tile_pool(name="psA", bufs=4, space="PSUM") as psA,
            tc.tile_pool(name="psS", bufs=2, space="PSUM") as psS,
            tc.tile_pool(name="psT", bufs=2, space="PSUM") as psT,
        ):
            # ---- load constants (x0 first: biggest + on critical path)
            x0 = big.tile([128, NG], F16, tag="s16a")
            nc.sync.dma_start(x0[:], x0T)
            wp = cst.tile([128, 1056], F16)
            nc.sync.dma_start(wp[:], wpack)
            bp = cst.tile([128, 8], F32)
            nc.sync.dma_start(bp[:], bpack)
            ob = cst.tile([1, 1], F32)
            nc.sync.dma_start(ob[:], outb)
            emb = cst.tile([128, N], F16)
            nc.sync.dma_start(emb[:], embT)
            oidx = cst.tile([128, NOUT // 16], I16)
            nc.sync.dma_start(oidx[:], outidx_d)
            ocnt = big.tile([128, NOUT], F16, tag="cnt")
            nc.sync.dma_start(ocnt[:], outcnt_d)
            sidx = cst.tile([128, 8 * Kout], I16)
            nc.sync.dma_start(sidx[:], scatidx_d)

            def bias(col):
                return bp[:, col:col + 1]

            # warm up the collective path early (absorbs setup/skew)
            warm = sm.tile([128, 2], F32)
            nc.vector.memset(warm[:], 0.0)
            nc.sync.dma_start(cc_win, warm[:])
            nc.gpsimd.collective_compute(
                "AllReduce", ALU.add,
                replica_groups=[list(range(n_cores))],
                ins=[cc_win], outs=[cc_wout])

            # ---- B: xT = lin_w @ x0T  (scores depend on this -> first)
            xT = big.tile([128, NG], F16, tag="s16b")
            tT = big.tile([128, NG], F16, tag="tT")
            for h in range(NH):
                s = h * CH
                ps = psA.tile([128, CH], F32, tag="A")
                nc.tensor.matmul(ps[:], wp[:, W_LIN:W_LIN + 128],
                                 x0[:, s:s + CH], start=True, stop=True)
                nc.vector.tensor_copy(xT[:, s:s + CH], ps[:])

            # ---- D: node scores -> sNN [16, 1024]: rows 0-7 s_i, 8-15 s_j
            sNN = sm.tile([16, 1024], F32)
            nc.vector.memset(sNN[:], 0.0)
            emsc = sm.tile([2, N], F32)
            for h in range(2):
                ps = psS.tile([2, CH], F32, tag="S")
                nc.tensor.matmul(ps[:], wp[:, W_ATTEM:W_ATTEM + 2],
                                 emb[:, h * CH:(h + 1) * CH], start=True, stop=True)
                nc.vector.tensor_copy(emsc[:, h * CH:(h + 1) * CH], ps[:])
            for h in range(NH):
                s = h * CH
                g, off = divmod(s, 1000)
                ps = psS.tile([2, CH], F32, tag="S")
                nc.tensor.matmul(ps[:], wp[:, W_ATTQ:W_ATTQ + 2],
                                 xT[:, s:s + CH], start=True, stop=True)
                st = stg.tile([2, CH], F32, tag="sc")
                # add tiled emb scores while still [2, CH]
                nc.vector.tensor_tensor(st[:], ps[:], emsc[:, off:off + CH],
                                        op=ALU.add)
                nc.sync.dma_start(
                    sNN[:, off:off + CH].rearrange("(a g) f -> g a f", a=2)[g],
                    st[:, :])

            if dbg:
                nc.sync.dma_start(d_siN, sNN[0:8, :])
                nc.sync.dma_start(d_sjN, sNN[8:16, :])

            # ---- E: si table (for src gather) + chunk-local sj
            sjC = sm.tile([128, 128], F32)
            siT = sm.tile([128, 1024], F32)  # row 16c+g: s_i[g, :] (+0 sentinel)
            nc.vector.memset(siT[:], 0.0)
            for g in range(G):
                nc.sync.dma_start(sjC[g::16, :], sNN[8 + g:9 + g, :].rearrange(
                    "p (c f) -> p c f", c=8))
                bci = sNN[g:g + 1, 0:N].unsqueeze(1).broadcast_to([1, 8, N])
                nc.sync.dma_start(siT[g::16, 0:N], bci)
            NCH = 4
            NODC = 128 // NCH

            # ---- C: tT = v_w @ x0T + v_b (off critical path)
            for h in range(NH):
                s = h * CH
                ps2 = psA.tile([128, CH], F32, tag="A")
                nc.tensor.matmul(ps2[:], wp[:, W_V:W_V + 128],
                                 x0[:, s:s + CH], start=True, stop=True)
                nc.scalar.activation(tT[:, s:s + CH], ps2[:], AF.Identity,
                                     bias=bias(B_VB))

            # ---- G: src-phase -> unnormalized scatter values cw (chunked)
            g2 = big.tile([128, NOUT], F32, tag="gth")
            cw = big.tile([128, NOUT], F16, tag="cw")
            for q in range(NCH):
                sl = slice(q * NODC * Kout, (q + 1) * NODC * Kout)
                isl = slice(q * NODC * Kout // 16, (q + 1) * NODC * Kout // 16)
                nc.gpsimd.ap_gather(g2[:, sl], siT[:], oidx[:, isl], channels=128,
                                    num_elems=1024, d=1, num_idxs=NODC * Kout)
                nc.vector.tensor_tensor(
                    cw[:, sl].rearrange("p (n k) -> p n k", k=Kout),
                    g2[:, sl].rearrange("p (j k) -> p j k", k=Kout),
                    sjC[:, q * NODC:(q + 1) * NODC].unsqueeze(2).broadcast_to(
                        [128, NODC, Kout]),
                    op=ALU.add)
                nc.vector.scalar_tensor_tensor(cw[:, sl], cw[:, sl], NEG, cw[:, sl],
                                               op0=ALU.mult, op1=ALU.max)
                nc.scalar.activation(cw[:, sl], cw[:, sl], AF.Exp)
                nc.vector.tensor_tensor(cw[:, sl], cw[:, sl], ocnt[:, sl],
                                        op=ALU.mult)
            if dbg:
                nc.sync.dma_start(d_cw, cw[:])

            # ---- H: x_nm tiles (transpose xT per graph-tile) for agg lhsT
            xnm = big.tile([128, 64 * 128], F16, tag="s16a")
            for g in range(G):
                for t in range(8):
                    w = 128 if t < 7 else 104
                    pt = psT.tile([128, 128], F16, tag="T")
                    nc.tensor.transpose(pt[0:w, :],
                                        xT[:, g * 1000 + t * 128:
                                           g * 1000 + t * 128 + w],
                                        wp[:, W_ID:W_ID + 128])
                    nc.vector.tensor_copy(
                        xnm[0:w, (g * 8 + t) * 128:(g * 8 + t) * 128 + 128],
                        pt[0:w, :])

            # ---- I: per-graph W build + agg & den matmuls, normalize on evict
            aggT = big.tile([128, NG], F16, tag="agg")
            sqs = big.tile([128, NG], F16, tag="gth")
            sumacc = sm.tile([128, 8], F32)
            sqacc = sm.tile([128, 8], F32)
            for g in range(G):
                WT = wtp.tile([128, 8000], F16, tag="wt")
                wdata = wdp.tile([128, 8 * Kout], F16, tag="wd")
                for c in range(8):
                    nc.sync.dma_start(
                        wdata[:, c * Kout:(c + 1) * Kout],
                        cw[16 * c + g:16 * c + g + 1, :].rearrange(
                            "p (j k) -> p j k", k=Kout))
                for t in range(8):
                    nc.gpsimd.local_scatter(
                        WT[:, t * 1000:(t + 1) * 1000],
                        wdata[:, t * Kout:(t + 1) * Kout],
                        sidx[:, t * Kout:(t + 1) * Kout],
                        channels=128, num_elems=N, num_idxs=Kout)
                if dbg and g == G - 1:
                    nc.sync.dma_start(d_wt, WT[:])
                denR16 = dnp.tile([1, 1024], F16, tag="dn")
                for hf in range(2):
                    pd = psS.tile([2, CH], F32, tag="S")
                    for t in range(8):
                        kt = 128 if t < 7 else 104
                        nc.tensor.matmul(
                            pd[0:1, :], wp[0:kt, W_ONE:W_ONE + 1],
                            WT[0:kt, t * 1000 + hf * CH:t * 1000 + hf * CH + CH],
                            start=(t == 0), stop=(t == 7))
                    nc.vector.tensor_copy(denR16[0:1, hf * CH:hf * CH + CH],
                                          pd[0:1, :])
                # denFull = ones ⊗ den, then parallel reciprocal on 128 lanes
                rdf = wdp.tile([128, 1024], F32, tag="rdf")
                for hf in range(2):
                    pr = psA.tile([128, CH], F32, tag="A")
                    nc.tensor.matmul(pr[:], wp[0:1, W_ONER:W_ONER + 128],
                                     denR16[0:1, hf * CH:hf * CH + CH],
                                     start=True, stop=True)
                    nc.vector.reciprocal(rdf[:, hf * CH:hf * CH + CH], pr[:])
                for hf in range(2):
                    pa = psA.tile([128, CH], F32, tag="A")
                    for t in range(8):
                        kt = 128 if t < 7 else 104
                        nc.tensor.matmul(
                            pa[:], xnm[0:kt, (g * 8 + t) * 128:
                                       (g * 8 + t) * 128 + 128],
                            WT[0:kt, t * 1000 + hf * CH:t * 1000 + hf * CH + CH],
                            start=(t == 0), stop=(t == 7))
                    nc.vector.scalar_tensor_tensor(
                        aggT[:, g * 1000 + hf * CH:g * 1000 + hf * CH + CH],
                        pa[:], 1.0, rdf[:, hf * CH:hf * CH + CH],
                        op0=ALU.mult, op1=ALU.mult)
                # per-graph BN partial sums on ACT (idle during the loop)
                nc.scalar.activation(sqs[:, g * 1000:(g + 1) * 1000],
                                     aggT[:, g * 1000:(g + 1) * 1000],
                                     AF.Identity, accum_out=sumacc[:, g:g + 1])
                nc.scalar.activation(sqs[:, g * 1000:(g + 1) * 1000],
                                     aggT[:, g * 1000:(g + 1) * 1000],
                                     AF.Square, accum_out=sqacc[:, g:g + 1])
            # ---- J/K: BN stats (gnn_bias folded analytically) + AllReduce
            stats = sm.tile([128, 2], F32)
            s1u = sm.tile([128, 4], F32)
            nc.vector.tensor_reduce(s1u[:, 0:1], sumacc[:],
                                    axis=mybir.AxisListType.X, op=ALU.add)
            nc.vector.tensor_reduce(s1u[:, 1:2], sqacc[:],
                                    axis=mybir.AxisListType.X, op=ALU.add)
            # stats0 = sum(u*r) + Nn*gb ; stats1 = sum((u*r)^2) + 2*gb*sum(u*r) + Nn*gb^2
            gb = bias(B_GNN)
            nc.vector.tensor_scalar(s1u[:, 2:3], gb, float(B * N), None, op0=ALU.mult)
            nc.vector.tensor_tensor(stats[:, 0:1], s1u[:, 0:1], s1u[:, 2:3],
                                    op=ALU.add)
            nc.vector.tensor_tensor(s1u[:, 3:4], s1u[:, 2:3], gb, op=ALU.mult)
            # t = 2*s1 + Nn*gb ; stats1 = t*gb + sum_sq
            nc.vector.scalar_tensor_tensor(stats[:, 1:2], s1u[:, 0:1], 2.0,
                                           s1u[:, 2:3], op0=ALU.mult, op1=ALU.add)
            nc.vector.tensor_tensor(stats[:, 1:2], stats[:, 1:2], gb, op=ALU.mult)
            nc.vector.tensor_tensor(stats[:, 1:2], stats[:, 1:2], s1u[:, 1:2],
                                    op=ALU.add)
            if dbg:
                nc.sync.dma_start(d_stats, stats[:])
            nc.sync.dma_start(cc_in, stats[:])
            nc.gpsimd.collective_compute(
                "AllReduce", ALU.add,
                replica_groups=[list(range(n_cores))],
                ins=[cc_in], outs=[cc_out])
            gstats = sm.tile([128, 2], F32)
            nc.sync.dma_start(gstats[:], cc_out)
            if dbg:
                nc.sync.dma_start(d_gstats, gstats[:])

            # ---- L: BN coefficients A, Bv
            cf = sm.tile([128, 8], F32)
            mu, msq, var, rsd, A_, Bv = (cf[:, i:i + 1] for i in range(6))
            inv_n = 1.0 / (B * N)
            nc.vector.tensor_scalar_mul(mu, gstats[:, 0:1], inv_n)
            nc.vector.tensor_scalar_mul(msq, gstats[:, 1:2], inv_n)
            nc.vector.tensor_tensor(var, mu, mu, op=ALU.mult)
            nc.vector.tensor_sub(var, msq, var)
            nc.scalar.activation(var, var, AF.Sqrt, bias=bias(B_EPS))
            nc.vector.reciprocal(rsd, var)
            nc.vector.tensor_tensor(A_, bias(B_GAM), rsd, op=ALU.mult)
            nc.vector.tensor_tensor(Bv, mu, A_, op=ALU.mult)
            nc.vector.tensor_sub(Bv, bias(B_BET), Bv)
            nc.vector.tensor_tensor(cf[:, 6:7], bias(B_GNN), A_, op=ALU.mult)
            nc.vector.tensor_tensor(Bv, Bv, cf[:, 6:7], op=ALU.add)

            if dbg:
                nc.sync.dma_start(d_cf, cf[:])

            # ---- M: s_out = relu(A*agg + Bv) (in place; split ACT/DVE)
            HF2 = NG // 2
            nc.scalar.activation(aggT[:, 0:HF2], aggT[:, 0:HF2], AF.Relu,
                                 bias=Bv, scale=A_)
            nc.vector.tensor_scalar(aggT[:, HF2:NG], aggT[:, HF2:NG], A_, Bv,
                                    op0=ALU.mult, op1=ALU.add)
            nc.vector.tensor_scalar_max(aggT[:, HF2:NG], aggT[:, HF2:NG], 0.0)

            # ---- N/O/P: fusion MLP + out layer
            hT = big.tile([128, NG], F16, tag="s16a")
            for h in range(NH):
                s = h * CH
                ps = psA.tile([128, CH], F32, tag="A")
                nc.tensor.matmul(ps[:], wp[:, W_F1:W_F1 + 128],
                                 aggT[:, s:s + CH], start=True, stop=False)
                nc.tensor.matmul(ps[:], wp[:, W_F1 + 128:W_F1 + 256],
                                 tT[:, s:s + CH], start=False, stop=True)
                if h % 2 == 0:
                    nc.scalar.activation(hT[:, s:s + CH], ps[:], AF.Relu,
                                         bias=bias(B_FB1))
                else:
                    nc.vector.tensor_scalar(hT[:, s:s + CH], ps[:], bias(B_FB1),
                                            None, op0=ALU.add)
                    nc.vector.tensor_scalar_max(hT[:, s:s + CH], hT[:, s:s + CH],
                                                0.0)
            if dbg:
                nc.sync.dma_start(d_hT, hT[:])
            # composite head: c = f_w2 @ out_w ; cb = <out_w, f_b2> + out_b
            cvec = sm.tile([128, 2], F16)
            cb = sm.tile([1, 2], F32)
            nc.vector.tensor_copy(cvec[:, 1:2], bias(B_FB2))  # f_b2 -> f16
            pc = psS.tile([2, 1], F32, tag="S")
            nc.tensor.matmul(pc[0:1, 0:1], cvec[:, 1:2],
                             wp[:, W_OUT:W_OUT + 1], start=True, stop=True)
            pc2 = psA.tile([128, CH], F32, tag="A")
            nc.tensor.matmul(pc2[:, 0:1], wp[:, W_F2P:W_F2P + 128],
                             wp[:, W_OUT:W_OUT + 1], start=True, stop=True)
            nc.vector.tensor_copy(cvec[:, 0:1], pc2[:, 0:1])
            nc.vector.tensor_copy(cb[:, 0:1], pc[0:1, 0:1])
            nc.vector.tensor_tensor(cb[:, 1:2], cb[:, 0:1], ob[:], op=ALU.add)
            for h in range(NH):
                s = h * CH
                ps = psS.tile([2, CH], F32, tag="S")
                nc.tensor.matmul(ps[0:1, :], cvec[:, 0:1],
                                 hT[:, s:s + CH], start=True, stop=True)
                yst = stg.tile([2, CH], F32, tag="sc")
                nc.scalar.activation(yst[0:1, :], ps[0:1, :], AF.Identity,
                                     bias=cb[:, 1:2])
                nc.sync.dma_start(y_out[:, s:s + CH], yst[0:1, :])

    nc.compile()
    return nc


# ---------------------------------------------------------------- entry point
def _prepare(inputs):
    """Returns (nc, in_maps) — host prep + cached module build."""
    data = np.asarray(inputs["data"], np.float32)
    edge_index = np.asarray(inputs["edge_index"])

    pre = _prep_indices(edge_index)
    Kout = pre["Kout"]

    key = (Kout,)
    if key not in _CACHE:
        _CACHE[key] = _build(Kout, M)
    nc = _CACHE[key]

    f16 = np.float16

    def t16(a):  # transpose [r, c] -> [c, r] f16 contiguous
        return np.ascontiguousarray(np.asarray(a, np.float32).T).astype(f16)

    wpack = np.zeros((128, 1056), f16)
    wpack[:, 0:128] = t16(inputs["lin_w"])
    wpack[:, 128:256] = t16(inputs["v_w"])
    wpack[:, 256:384] = t16(inputs["f_w2"])
    wpack[:, 384:640] = np.ascontiguousarray(
        np.asarray(inputs["f_w1"], np.float32).T).astype(f16).reshape(2, 128, 128
        ).transpose(1, 0, 2).reshape(128, 256)
    wpack[:, 640] = np.asarray(inputs["att_i"], np.float32).astype(f16)
    wpack[:, 641] = np.asarray(inputs["att_j"], np.float32).astype(f16)
    wpack[:, 642] = np.asarray(inputs["att_em_i"], np.float32).astype(f16)
    wpack[:, 643] = np.asarray(inputs["att_em_j"], np.float32).astype(f16)
    wpack[:, 644] = np.asarray(inputs["out_w"], np.float32)[0].astype(f16)
    wpack[:, 645:773] = np.eye(128, dtype=f16)
    wpack[:, 773] = 1.0
    wpack[0, 774:902] = 1.0
    wpack[:, 902:1030] = np.asarray(inputs["f_w2"], np.float32).astype(f16)

    bpack = np.zeros((128, 8), np.float32)
    bpack[:, 0] = np.asarray(inputs["v_b"], np.float32)
    bpack[:, 1] = np.asarray(inputs["gnn_bias"], np.float32)
    bpack[:, 2] = np.asarray(inputs["f_b1"], np.float32)
    bpack[:, 3] = np.asarray(inputs["f_b2"], np.float32)
    bpack[:, 4] = np.asarray(inputs["bn_gamma"], np.float32)
    bpack[:, 5] = np.asarray(inputs["bn_beta"], np.float32)
    bpack[:, 6] = EPS
    outb = np.asarray(inputs["out_b"], np.float32).reshape(1, 1)

    embT = t16(inputs["emb"])

    shared = dict(
        embT=embT, wpack=wpack, bpack=bpack, outb=outb,
        outidx=pre["outidx"], outcnt=pre["outcnt"], scatidx=pre["scatidx"],
    )
    in_maps = []
    for d in range(M):
        x0T = np.ascontiguousarray(
            data[d * G:(d + 1) * G].transpose(2, 0, 1).reshape(128, NG)
        ).astype(f16)
        in_maps.append(dict(shared, x0T=x0T))
    return nc, in_maps


def kernel(**inputs):
    nc, in_maps = _prepare(inputs)
    res = run_bass_kernel_spmd(nc, in_maps, list(range(M)))
    out = np.empty(B * N, np.float32)
    for d in range(M):
        out[d * NG:(d + 1) * NG] = res.results[d]["y"].reshape(-1)
    return out


# revision 26
# speedup vs baseline: 1.0091x; 1.0091x over previous
"""EnhancedGDN Trainium2 kernel.

Data-parallel over batch B=64 across 8 NeuronCores (8 graphs each).
All 64 graphs share one edge list -> host does integer-only index prep
(dedup + CSR padding); all float math runs on device.

Math (per device, 8 graphs):
  t_out = data @ v_w.T + v_b                  (temporal attn: window=1 -> softmax==1)
  x     = data @ lin_w.T
  s_i/s_j per node from att vectors (+ tiled emb scores)
  per-edge: alpha = lrelu(s_i[dst] + s_j[src]); softmax over in-edges of dst
  agg   = sum_e w_e * x[src]  (dense per-graph W matmul; W built by local_scatter)
  BatchNorm over all 64k nodes (AllReduce of sums) + ReLU
  out   = (relu([s_out|t_out] @ f_w1.T + f_b1) @ f_w2.T + f_b2) @ out_w.T + out_b
"""

import os

os.environ.setdefault("NEURON_RT_RESET_CORES", "1")

import numpy as np

import concourse.bass as bass
import concourse.bacc as bacc
import concourse.tile as tile
from concourse import mybir
from concourse.bass_utils import run_bass_kernel_spmd

B, N, D, E = 64, 1000, 128, 20000
M = 8          # devices
G = B // M     # graphs per device
NG = G * N     # nodes per device
NEG = 0.2
EPS = 1e-5

F16 = mybir.dt.float16
F32 = mybir.dt.float32
I16 = mybir.dt.int16
AF = mybir.ActivationFunctionType
ALU = mybir.AluOpType

_CACHE = {}


# ---------------------------------------------------------------- host index prep
def _prep_indices(edge_index):
    src = edge_index[0].astype(np.int64)
    dst = edge_index[1].astype(np.int64)
    key = dst * N + src
    uniq, cnt = np.unique(key, return_counts=True)
    ii = uniq // N
    jj = uniq % N
    # add self loops (reference removes none exist, then adds them)
    ii = np.concatenate([ii, np.arange(N)])
    jj = np.concatenate([jj, np.arange(N)])
    cc = np.concatenate([cnt, np.ones(N, np.int64)]).astype(np.float32)

    def _round4(k):
        return max(4, (k + 3) // 4 * 4)


    # out-CSR (grouped by src j)
    order = np.argsort(jj, kind="stable")
    oj, oi, oc = jj[order], ii[order], cc[order]
    outdeg = np.bincount(oj, minlength=N)
    Kout = _round4(int(outdeg.max()))
    out_i = np.full((1024, Kout), 1000, np.int64)  # gather pads -> sentinel
    out_i_sc = np.full((1024, Kout), -1, np.int64)  # -1 pads for local_scatter
    out_c = np.zeros((1024, Kout), np.float32)
    starts = np.searchsorted(oj, np.arange(N))
    kpos = np.arange(len(oj)) - starts[oj]
    out_i[oj, kpos] = oi
    out_i_sc[oj, kpos] = oi
    out_c[oj, kpos] = oc

    def _wrap(flat_per_core):
        # flat_per_core: [8, 128*K] -> wrapped [128, 128*K//16]
        nidx = flat_per_core.shape[1]
        w = np.zeros((128, nidx // 16), np.int16)
        for c in range(8):
            for p in range(16):
                w[16 * c + p] = flat_per_core[c, p::16]
        return w

    out_flat = out_i.reshape(8, 128 * Kout)       # core c -> src chunk c
    outidx = _wrap(out_flat)
    outcnt = np.repeat(out_c.reshape(8, 128 * Kout), 16, axis=0).astype(np.float16)
    scatidx = out_i_sc.reshape(8, 128, Kout).transpose(1, 0, 2).reshape(128, 8 * Kout)
    scatidx = np.ascontiguousarray(scatidx).astype(np.int16)
    return dict(Kout=Kout, outidx=outidx, outcnt=outcnt, scatidx=scatidx)


# ---------------------------------------------------------------- device module
def _build(Kout, n_cores, dbg=False):
    NOUT = 128 * Kout   # per-core src-phase slots
    NH = 16             # eviction chunks of 500 over 8000
    CH = 500

    nc = bacc.Bacc("TRN2", target_bir_lowering=False, debug=False,
                   num_devices=n_cores)

    def din(name, shape, dt):
        return nc.dram_tensor(name, shape, dt, kind="ExternalInput").ap()

    x0T = din("x0T", [128, NG], F16)
    embT = din("embT", [128, N], F16)
    wpack = din("wpack", [128, 1184], F16)
    bpack = din("bpack", [128, 8], F32)
    outb = din("outb", [1, 1], F32)
    outidx_d = din("outidx", [128, NOUT // 16], I16)
    outcnt_d = din("outcnt", [128, NOUT], F16)
    scatidx_d = din("scatidx", [128, 8 * Kout], I16)
    y_out = nc.dram_tensor("y", [1, NG], F32, kind="ExternalOutput").ap()

    if dbg:
        dbg_outs = {}
        def dout(name, shape, dt):
            ap = nc.dram_tensor(name, shape, dt, kind="ExternalOutput").ap()
            dbg_outs[name] = ap
            return ap
        d_siN = dout("d_siN", [8, 1024], F32)
        d_sjN = dout("d_sjN", [8, 1024], F32)
        d_cw = dout("d_cw", [128, 128 * Kout], F16)
        d_agg = dout("d_agg", [128, NG], F16)
        d_stats = dout("d_stats", [128, 2], F32)
        d_gstats = dout("d_gstats", [128, 2], F32)
        d_cf = dout("d_cf", [128, 6], F32)
        d_xT = dout("d_xT", [128, NG], F16)
        d_tT = dout("d_tT", [128, NG], F16)
        d_hT = dout("d_hT", [128, NG], F16)
        d_wt = dout("d_wt", [128, 8000], F16)

    cc_in = nc.dram_tensor("cc_in", [128, 2], F32).ap()
    cc_out = nc.dram_tensor("cc_out", [128, 2], F32, addr_space="Shared").ap()
    cc_win = nc.dram_tensor("cc_win", [128, 2], F32).ap()
    cc_wout = nc.dram_tensor("cc_wout", [128, 2], F32, addr_space="Shared").ap()

    # wpack column layout
    W_LIN, W_V, W_F2, W_F1, W_ATTQ, W_ATTEM, W_OUT, W_ID, W_ONE = (
        0, 128, 256, 384, 640, 642, 644, 645, 773)
    W_ONER = 774
    W_F2P = 902
    W_LINP = 1030
    # bpack columns: v_b, gnn_bias, f_b1, f_b2, bn_gamma, bn_beta
    B_VB, B_GNN, B_FB1, B_FB2, B_GAM, B_BET, B_EPS = 0, 1, 2, 3, 4, 5, 6

    with tile.TileContext(nc) as tc:
        with (
            tc.tile_pool(name="cst", bufs=1) as cst,
            tc.tile_pool(name="big", bufs=1) as big,
            tc.tile_pool(name="wt", bufs=3) as wtp,
            tc.tile_pool(name="sm", bufs=1) as sm,
            tc.tile_pool(name="stg", bufs=4) as stg,
            tc.tile_pool(name="wd", bufs=2) as wdp,
            tc.tile_pool(name="dnp", bufs=2) as dnp,
            tc.tile_pool(name="psA", bufs=4, space="PSUM") as psA,
            tc.tile_pool(name="psS", bufs=2, space="PSUM") as psS,
            tc.tile_pool(name="psT", bufs=2, space="PSUM") as psT,
        ):
            # ---- load constants (x0 first: biggest + on critical path)
            x0 = big.tile([128, NG], F16, tag="s16a")
            nc.sync.dma_start(x0[:], x0T)
            wp = cst.tile([128, 1184], F16)
            nc.sync.dma_start(wp[:], wpack)
            bp = cst.tile([128, 8], F32)
            nc.sync.dma_start(bp[:], bpack)
            ob = cst.tile([1, 1], F32)
            nc.sync.dma_start(ob[:], outb)
            emb = cst.tile([128, N], F16)
            nc.sync.dma_start(emb[:], embT)
            oidx = cst.tile([128, NOUT // 16], I16)
            nc.sync.dma_start(oidx[:], outidx_d)
            ocnt = big.tile([128, NOUT], F16, tag="cnt")
            nc.sync.dma_start(ocnt[:], outcnt_d)
            sidx = cst.tile([128, 8 * Kout], I16)
            nc.sync.dma_start(sidx[:], scatidx_d)

            def bias(col):
                return bp[:, col:col + 1]

            # warm up the collective path early (absorbs setup/skew)
            warm = sm.tile([128, 2], F32)
            nc.vector.memset(warm[:], 0.0)
            nc.sync.dma_start(cc_win, warm[:])
            nc.gpsimd.collective_compute(
                "AllReduce", ALU.add,
                replica_groups=[list(range(n_cores))],
                ins=[cc_win], outs=[cc_wout])

            # ---- B: xT = lin_w @ x0T  (scores depend on this -> first)
            xT = big.tile([128, NG], F16, tag="s16b")
            tT = big.tile([128, NG], F16, tag="tT")
            for h in range(NH):
                s = h * CH
                ps = psA.tile([128, CH], F32, tag="A")
                nc.tensor.matmul(ps[:], wp[:, W_LIN:W_LIN + 128],
                                 x0[:, s:s + CH], start=True, stop=True)
                nc.vector.tensor_copy(xT[:, s:s + CH], ps[:])

            # ---- D: node scores -> sNN [16, 1024]: rows 0-7 s_i, 8-15 s_j
            sNN = sm.tile([16, 1024], F32)
            nc.vector.memset(sNN[:], 0.0)
            emsc = sm.tile([2, N], F32)
            for h in range(2):
                ps = psS.tile([2, CH], F32, tag="S")
                nc.tensor.matmul(ps[:], wp[:, W_ATTEM:W_ATTEM + 2],
                                 emb[:, h * CH:(h + 1) * CH], start=True, stop=True)
                nc.vector.tensor_copy(emsc[:, h * CH:(h + 1) * CH], ps[:])
            attc = sm.tile([128, 2], F16)
            pat = psA.tile([128, CH], F32, tag="A")
            nc.tensor.matmul(pat[:, 0:2], wp[:, W_LINP:W_LINP + 128],
                             wp[:, W_ATTQ:W_ATTQ + 2], start=True, stop=True)
            nc.vector.tensor_copy(attc[:], pat[:, 0:2])
            for h in range(NH):
                s = h * CH
                g, off = divmod(s, 1000)
                ps = psS.tile([2, CH], F32, tag="S")
                nc.tensor.matmul(ps[:], attc[:, 0:2],
                                 x0[:, s:s + CH], start=True, stop=True)
                st = stg.tile([2, CH], F32, tag="sc")
                # add tiled emb scores while still [2, CH]
                nc.vector.tensor_tensor(st[:], ps[:], emsc[:, off:off + CH],
                                        op=ALU.add)
                nc.sync.dma_start(
                    sNN[:, off:off + CH].rearrange("(a g) f -> g a f", a=2)[g],
                    st[:, :])

            if dbg:
                nc.sync.dma_start(d_siN, sNN[0:8, :])
                nc.sync.dma_start(d_sjN, sNN[8:16, :])

            # ---- E: si table (for src gather) + chunk-local sj
            sjC = sm.tile([128, 128], F32)
            siT = sm.tile([128, 1024], F32)  # row 16c+g: s_i[g, :] (+0 sentinel)
            nc.vector.memset(siT[:], 0.0)
            for g in range(G):
                nc.sync.dma_start(sjC[g::16, :], sNN[8 + g:9 + g, :].rearrange(
                    "p (c f) -> p c f", c=8))
                bci = sNN[g:g + 1, 0:N].unsqueeze(1).broadcast_to([1, 8, N])
                nc.sync.dma_start(siT[g::16, 0:N], bci)
            NCH = 4
            NODC = 128 // NCH

            # ---- C: tT = v_w @ x0T + v_b (off critical path)
            for h in range(NH):
                s = h * CH
                ps2 = psA.tile([128, CH], F32, tag="A")
                nc.tensor.matmul(ps2[:], wp[:, W_V:W_V + 128],
                                 x0[:, s:s + CH], start=True, stop=True)
                nc.scalar.activation(tT[:, s:s + CH], ps2[:], AF.Identity,
                                     bias=bias(B_VB))

            # ---- G: src-phase -> unnormalized scatter values cw (chunked)
            g2 = big.tile([128, NOUT], F32, tag="gth")
            cw = big.tile([128, NOUT], F16, tag="cw")
            for q in range(NCH):
                sl = slice(q * NODC * Kout, (q + 1) * NODC * Kout)
                isl = slice(q * NODC * Kout // 16, (q + 1) * NODC * Kout // 16)
                nc.gpsimd.ap_gather(g2[:, sl], siT[:], oidx[:, isl], channels=128,
                                    num_elems=1024, d=1, num_idxs=NODC * Kout)
                nc.vector.tensor_tensor(
                    cw[:, sl].rearrange("p (n k) -> p n k", k=Kout),
                    g2[:, sl].rearrange("p (j k) -> p j k", k=Kout),
                    sjC[:, q * NODC:(q + 1) * NODC].unsqueeze(2).broadcast_to(
                        [128, NODC, Kout]),
                    op=ALU.add)
                nc.vector.scalar_tensor_tensor(cw[:, sl], cw[:, sl], NEG, cw[:, sl],
                                               op0=ALU.mult, op1=ALU.max)
                nc.scalar.activation(cw[:, sl], cw[:, sl], AF.Exp)
                nc.vector.tensor_tensor(cw[:, sl], cw[:, sl], ocnt[:, sl],
                                        op=ALU.mult)
            if dbg:
                nc.sync.dma_start(d_cw, cw[:])

            # ---- H: x_nm tiles (transpose xT per graph-tile) for agg lhsT
            xnm = big.tile([128, 64 * 128], F16, tag="s16a")
            for g in range(G):
                for t in range(8):
                    w = 128 if t < 7 else 104
                    pt = psT.tile([128, 128], F16, tag="T")
                    nc.tensor.transpose(pt[0:w, :],
                                        xT[:, g * 1000 + t * 128:
                                           g * 1000 + t * 128 + w],
                                        wp[:, W_ID:W_ID + 128])
                    nc.vector.tensor_copy(
                        xnm[0:w, (g * 8 + t) * 128:(g * 8 + t) * 128 + 128],
                        pt[0:w, :])

            # ---- I: per-graph W build + agg & den matmuls, normalize on evict
            aggT = big.tile([128, NG], F16, tag="agg")
            sqs = big.tile([128, NG], F16, tag="gth")
            sumacc = sm.tile([128, 8], F32)
            sqacc = sm.tile([128, 8], F32)
            for g in range(G):
                WT = wtp.tile([128, 8000], F16, tag="wt")
                wdata = wdp.tile([128, 8 * Kout], F16, tag="wd")
                for c in range(8):
                    nc.sync.dma_start(
                        wdata[:, c * Kout:(c + 1) * Kout],
                        cw[16 * c + g:16 * c + g + 1, :].rearrange(
                            "p (j k) -> p j k", k=Kout))
                for t in range(8):
                    nc.gpsimd.local_scatter(
                        WT[:, t * 1000:(t + 1) * 1000],
                        wdata[:, t * Kout:(t + 1) * Kout],
                        sidx[:, t * Kout:(t + 1) * Kout],
                        channels=128, num_elems=N, num_idxs=Kout)
                if dbg and g == G - 1:
                    nc.sync.dma_start(d_wt, WT[:])
                denR16 = dnp.tile([1, 1024], F16, tag="dn")
                for hf in range(2):
                    pd = psS.tile([2, CH], F32, tag="S")
                    for t in range(8):
                        kt = 128 if t < 7 else 104
                        nc.tensor.matmul(
                            pd[0:1, :], wp[0:kt, W_ONE:W_ONE + 1],
                            WT[0:kt, t * 1000 + hf * CH:t * 1000 + hf * CH + CH],
                            start=(t == 0), stop=(t == 7))
                    nc.vector.tensor_copy(denR16[0:1, hf * CH:hf * CH + CH],
                                          pd[0:1, :])
                # denFull = ones ⊗ den, then parallel reciprocal on 128 lanes
                rdf = wdp.tile([128, 1024], F32, tag="rdf")
                for hf in range(2):
                    pr = psA.tile([128, CH], F32, tag="A")
                    nc.tensor.matmul(pr[:], wp[0:1, W_ONER:W_ONER + 128],
                                     denR16[0:1, hf * CH:hf * CH + CH],
                                     start=True, stop=True)
                    nc.vector.reciprocal(rdf[:, hf * CH:hf * CH + CH], pr[:])
                for hf in range(2):
                    pa = psA.tile([128, CH], F32, tag="A")
                    for t in range(8):
                        kt = 128 if t < 7 else 104
                        nc.tensor.matmul(
                            pa[:], xnm[0:kt, (g * 8 + t) * 128:
                                       (g * 8 + t) * 128 + 128],
                            WT[0:kt, t * 1000 + hf * CH:t * 1000 + hf * CH + CH],
                            start=(t == 0), stop=(t == 7))
                    nc.vector.scalar_tensor_tensor(
                        aggT[:, g * 1000 + hf * CH:g * 1000 + hf * CH + CH],
                        pa[:], 1.0, rdf[:, hf * CH:hf * CH + CH],
                        op0=ALU.mult, op1=ALU.mult)
                # per-graph BN partial sums on ACT (idle during the loop)
                nc.scalar.activation(sqs[:, g * 1000:(g + 1) * 1000],
                                     aggT[:, g * 1000:(g + 1) * 1000],
                                     AF.Identity, accum_out=sumacc[:, g:g + 1])
                nc.scalar.activation(sqs[:, g * 1000:(g + 1) * 1000],
                                     aggT[:, g * 1000:(g + 1) * 1000],
                                     AF.Square, accum_out=sqacc[:, g:g + 1])
            # ---- J/K: BN stats (gnn_bias folded analytically) + AllReduce
            stats = sm.tile([128, 2], F32)
            s1u = sm.tile([128, 4], F32)
            nc.vector.tensor_reduce(s1u[:, 0:1], sumacc[:],
                                    axis=mybir.AxisListType.X, op=ALU.add)
            nc.vector.tensor_reduce(s1u[:, 1:2], sqacc[:],
                                    axis=mybir.AxisListType.X, op=ALU.add)
            # stats0 = sum(u*r) + Nn*gb ; stats1 = sum((u*r)^2) + 2*gb*sum(u*r) + Nn*gb^2
            gb = bias(B_GNN)
            nc.vector.tensor_scalar(s1u[:, 2:3], gb, float(B * N), None, op0=ALU.mult)
            nc.vector.tensor_tensor(stats[:, 0:1], s1u[:, 0:1], s1u[:, 2:3],
                                    op=ALU.add)
            nc.vector.tensor_tensor(s1u[:, 3:4], s1u[:, 2:3], gb, op=ALU.mult)
            # t = 2*s1 + Nn*gb ; stats1 = t*gb + sum_sq
            nc.vector.scalar_tensor_tensor(stats[:, 1:2], s1u[:, 0:1], 2.0,
                                           s1u[:, 2:3], op0=ALU.mult, op1=ALU.add)
            nc.vector.tensor_tensor(stats[:, 1:2], stats[:, 1:2], gb, op=ALU.mult)
            nc.vector.tensor_tensor(stats[:, 1:2], stats[:, 1:2], s1u[:, 1:2],
                                    op=ALU.add)
            if dbg:
                nc.sync.dma_start(d_stats, stats[:])
            nc.sync.dma_start(cc_in, stats[:])
            nc.gpsimd.collective_compute(
                "AllReduce", ALU.add,
                replica_groups=[list(range(n_cores))],
                ins=[cc_in], outs=[cc_out])
            gstats = sm.tile([128, 2], F32)
            nc.sync.dma_start(gstats[:], cc_out)
            if dbg:
                nc.sync.dma_start(d_gstats, gstats[:])

            # ---- L: BN coefficients A, Bv
            cf = sm.tile([128, 8], F32)
            mu, msq, var, rsd, A_, Bv = (cf[:, i:i + 1] for i in range(6))
            inv_n = 1.0 / (B * N)
            nc.vector.tensor_scalar_mul(mu, gstats[:, 0:1], inv_n)
            nc.vector.tensor_scalar_mul(msq, gstats[:, 1:2], inv_n)
            nc.vector.tensor_tensor(var, mu, mu, op=ALU.mult)
            nc.vector.tensor_sub(var, msq, var)
            nc.scalar.activation(var, var, AF.Sqrt, bias=bias(B_EPS))
            nc.vector.reciprocal(rsd, var)
            nc.vector.tensor_tensor(A_, bias(B_GAM), rsd, op=ALU.mult)
            nc.vector.tensor_tensor(Bv, mu, A_, op=ALU.mult)
            nc.vector.tensor_sub(Bv, bias(B_BET), Bv)
            nc.vector.tensor_tensor(cf[:, 6:7], bias(B_GNN), A_, op=ALU.mult)
            nc.vector.tensor_tensor(Bv, Bv, cf[:, 6:7], op=ALU.add)

            if dbg:
                nc.sync.dma_start(d_cf, cf[:])

            # ---- M: s_out = relu(A*agg + Bv) (in place; split ACT/DVE)
            HF2 = NG // 2
            nc.scalar.activation(aggT[:, 0:HF2], aggT[:, 0:HF2], AF.Relu,
                                 bias=Bv, scale=A_)
            nc.vector.tensor_scalar(aggT[:, HF2:NG], aggT[:, HF2:NG], A_, Bv,
                                    op0=ALU.mult, op1=ALU.add)
            nc.vector.tensor_scalar_max(aggT[:, HF2:NG], aggT[:, HF2:NG], 0.0)

            # ---- N/O/P: fusion MLP + out layer
            hT = big.tile([128, NG], F16, tag="s16a")
            for h in range(NH):
                s = h * CH
                ps = psA.tile([128, CH], F32, tag="A")
                nc.tensor.matmul(ps[:], wp[:, W_F1:W_F1 + 128],
                                 aggT[:, s:s + CH], start=True, stop=False)
                nc.tensor.matmul(ps[:], wp[:, W_F1 + 128:W_F1 + 256],
                                 tT[:, s:s + CH], start=False, stop=True)
                if h % 2 == 0:
                    nc.scalar.activation(hT[:, s:s + CH], ps[:], AF.Relu,
                                         bias=bias(B_FB1))
                else:
                    nc.vector.tensor_scalar(hT[:, s:s + CH], ps[:], bias(B_FB1),
                                            None, op0=ALU.add)
                    nc.vector.tensor_scalar_max(hT[:, s:s + CH], hT[:, s:s + CH],
                                                0.0)
            if dbg:
                nc.sync.dma_start(d_hT, hT[:])
            # composite head: c = f_w2 @ out_w ; cb = <out_w, f_b2> + out_b
            cvec = sm.tile([128, 2], F16)
            cb = sm.tile([1, 2], F32)
            nc.vector.tensor_copy(cvec[:, 1:2], bias(B_FB2))  # f_b2 -> f16
            pc = psS.tile([2, 1], F32, tag="S")
            nc.tensor.matmul(pc[0:1, 0:1], cvec[:, 1:2],
                             wp[:, W_OUT:W_OUT + 1], start=True, stop=True)
            pc2 = psA.tile([128, CH], F32, tag="A")
            nc.tensor.matmul(pc2[:, 0:1], wp[:, W_F2P:W_F2P + 128],
                             wp[:, W_OUT:W_OUT + 1], start=True, stop=True)
            nc.vector.tensor_copy(cvec[:, 0:1], pc2[:, 0:1])
            nc.vector.tensor_copy(cb[:, 0:1], pc[0:1, 0:1])
            nc.vector.tensor_tensor(cb[:, 1:2], cb[:, 0:1], ob[:], op=ALU.add)
            for h in range(NH):
                s = h * CH
                ps = psS.tile([2, CH], F32, tag="S")
                nc.tensor.matmul(ps[0:1, :], cvec[:, 0:1],
                                 hT[:, s:s + CH], start=True, stop=True)
                yst = stg.tile([2, CH], F32, tag="sc")
                nc.scalar.activation(yst[0:1, :], ps[0:1, :], AF.Identity,
                                     bias=cb[:, 1:2])
                nc.sync.dma_start(y_out[:, s:s + CH], yst[0:1, :])

    nc.compile()
    return nc


# ---------------------------------------------------------------- entry point
def _prepare(inputs):
    """Returns (nc, in_maps) — host prep + cached module build."""
    data = np.asarray(inputs["data"], np.float32)
    edge_index = np.asarray(inputs["edge_index"])

    pre = _prep_indices(edge_index)
    Kout = pre["Kout"]

    key = (Kout,)
    if key not in _CACHE:
        _CACHE[key] = _build(Kout, M)
    nc = _CACHE[key]

    f16 = np.float16

    def t16(a):  # transpose [r, c] -> [c, r] f16 contiguous
        return np.ascontiguousarray(np.asarray(a, np.float32).T).astype(f16)

    wpack = np.zeros((128, 1184), f16)
    wpack[:, 0:128] = t16(inputs["lin_w"])
    wpack[:, 128:256] = t16(inputs["v_w"])
    wpack[:, 256:384] = t16(inputs["f_w2"])
    wpack[:, 384:640] = np.ascontiguousarray(
        np.asarray(inputs["f_w1"], np.float32).T).astype(f16).reshape(2, 128, 128
        ).transpose(1, 0, 2).reshape(128, 256)
    wpack[:, 640] = np.asarray(inputs["att_i"], np.float32).astype(f16)
    wpack[:, 641] = np.asarray(inputs["att_j"], np.float32).astype(f16)
    wpack[:, 642] = np.asarray(inputs["att_em_i"], np.float32).astype(f16)
    wpack[:, 643] = np.asarray(inputs["att_em_j"], np.float32).astype(f16)
    wpack[:, 644] = np.asarray(inputs["out_w"], np.float32)[0].astype(f16)
    wpack[:, 645:773] = np.eye(128, dtype=f16)
    wpack[:, 773] = 1.0
    wpack[0, 774:902] = 1.0
    wpack[:, 902:1030] = np.asarray(inputs["f_w2"], np.float32).astype(f16)
    wpack[:, 1030:1158] = np.asarray(inputs["lin_w"], np.float32).astype(f16)

    bpack = np.zeros((128, 8), np.float32)
    bpack[:, 0] = np.asarray(inputs["v_b"], np.float32)
    bpack[:, 1] = np.asarray(inputs["gnn_bias"], np.float32)
    bpack[:, 2] = np.asarray(inputs["f_b1"], np.float32)
    bpack[:, 3] = np.asarray(inputs["f_b2"], np.float32)
    bpack[:, 4] = np.asarray(inputs["bn_gamma"], np.float32)
    bpack[:, 5] = np.asarray(inputs["bn_beta"], np.float32)
    bpack[:, 6] = EPS
    outb = np.asarray(inputs["out_b"], np.float32).reshape(1, 1)

    embT = t16(inputs["emb"])

    shared = dict(
        embT=embT, wpack=wpack, bpack=bpack, outb=outb,
        outidx=pre["outidx"], outcnt=pre["outcnt"], scatidx=pre["scatidx"],
    )
    in_maps = []
    for d in range(M):
        x0T = np.ascontiguousarray(
            data[d * G:(d + 1) * G].transpose(2, 0, 1).reshape(128, NG)
        ).astype(f16)
        in_maps.append(dict(shared, x0T=x0T))
    return nc, in_maps


def kernel(**inputs):
    nc, in_maps = _prepare(inputs)
    res = run_bass_kernel_spmd(nc, in_maps, list(range(M)))
    out = np.empty(B * N, np.float32)
    for d in range(M):
        out[d * NG:(d + 1) * NG] = res.results[d]["y"].reshape(-1)
    return out


# revision 28
# speedup vs baseline: 1.0166x; 1.0075x over previous
"""EnhancedGDN Trainium2 kernel.

Data-parallel over batch B=64 across 8 NeuronCores (8 graphs each).
All 64 graphs share one edge list -> host does integer-only index prep
(dedup + CSR padding); all float math runs on device.

Math (per device, 8 graphs):
  t_out = data @ v_w.T + v_b                  (temporal attn: window=1 -> softmax==1)
  x     = data @ lin_w.T
  s_i/s_j per node from att vectors (+ tiled emb scores)
  per-edge: alpha = lrelu(s_i[dst] + s_j[src]); softmax over in-edges of dst
  agg   = sum_e w_e * x[src]  (dense per-graph W matmul; W built by local_scatter)
  BatchNorm over all 64k nodes (AllReduce of sums) + ReLU
  out   = (relu([s_out|t_out] @ f_w1.T + f_b1) @ f_w2.T + f_b2) @ out_w.T + out_b
"""

import os

os.environ.setdefault("NEURON_RT_RESET_CORES", "1")

import numpy as np

import concourse.bass as bass
import concourse.bacc as bacc
import concourse.tile as tile
from concourse import mybir
from concourse.bass_utils import run_bass_kernel_spmd

B, N, D, E = 64, 1000, 128, 20000
M = 8          # devices
G = B // M     # graphs per device
NG = G * N     # nodes per device
NEG = 0.2
EPS = 1e-5

F16 = mybir.dt.float16
F32 = mybir.dt.float32
I16 = mybir.dt.int16
AF = mybir.ActivationFunctionType
ALU = mybir.AluOpType

_CACHE = {}


# ---------------------------------------------------------------- host index prep
def _prep_indices(edge_index):
    src = edge_index[0].astype(np.int64)
    dst = edge_index[1].astype(np.int64)
    key = dst * N + src
    uniq, cnt = np.unique(key, return_counts=True)
    ii = uniq // N
    jj = uniq % N
    # add self loops (reference removes none exist, then adds them)
    ii = np.concatenate([ii, np.arange(N)])
    jj = np.concatenate([jj, np.arange(N)])
    cc = np.concatenate([cnt, np.ones(N, np.int64)]).astype(np.float32)

    def _round4(k):
        return max(4, (k + 3) // 4 * 4)


    # out-CSR (grouped by src j)
    order = np.argsort(jj, kind="stable")
    oj, oi, oc = jj[order], ii[order], cc[order]
    outdeg = np.bincount(oj, minlength=N)
    Kout = _round4(int(outdeg.max()))
    out_i = np.full((1024, Kout), 1000, np.int64)  # gather pads -> sentinel
    out_i_sc = np.full((1024, Kout), -1, np.int64)  # -1 pads for local_scatter
    out_c = np.zeros((1024, Kout), np.float32)
    starts = np.searchsorted(oj, np.arange(N))
    kpos = np.arange(len(oj)) - starts[oj]
    out_i[oj, kpos] = oi
    out_i_sc[oj, kpos] = oi
    out_c[oj, kpos] = oc

    def _wrap(flat_per_core):
        # flat_per_core: [8, 128*K] -> wrapped [128, 128*K//16]
        nidx = flat_per_core.shape[1]
        w = np.zeros((128, nidx // 16), np.int16)
        for c in range(8):
            for p in range(16):
                w[16 * c + p] = flat_per_core[c, p::16]
        return w

    out_flat = out_i.reshape(8, 128 * Kout)       # core c -> src chunk c
    outidx = _wrap(out_flat)
    outcnt = np.repeat(out_c.reshape(8, 128 * Kout), 16, axis=0).astype(np.float16)
    scatidx = out_i_sc.reshape(8, 128, Kout).transpose(1, 0, 2).reshape(128, 8 * Kout)
    scatidx = np.ascontiguousarray(scatidx).astype(np.int16)
    return dict(Kout=Kout, outidx=outidx, outcnt=outcnt, scatidx=scatidx)


# ---------------------------------------------------------------- device module
def _build(Kout, n_cores, dbg=False):
    NOUT = 128 * Kout   # per-core src-phase slots
    NH = 16             # eviction chunks of 500 over 8000
    CH = 500

    nc = bacc.Bacc("TRN2", target_bir_lowering=False, debug=False,
                   num_devices=n_cores)

    def din(name, shape, dt):
        return nc.dram_tensor(name, shape, dt, kind="ExternalInput").ap()

    x0T = din("x0T", [128, NG], F16)
    embT = din("embT", [128, N], F16)
    wpack = din("wpack", [128, 1184], F16)
    bpack = din("bpack", [128, 8], F32)
    outb = din("outb", [1, 1], F32)
    outidx_d = din("outidx", [128, NOUT // 16], I16)
    outcnt_d = din("outcnt", [128, NOUT], F16)
    scatidx_d = din("scatidx", [128, 8 * Kout], I16)
    y_out = nc.dram_tensor("y", [1, NG], F32, kind="ExternalOutput").ap()

    if dbg:
        dbg_outs = {}
        def dout(name, shape, dt):
            ap = nc.dram_tensor(name, shape, dt, kind="ExternalOutput").ap()
            dbg_outs[name] = ap
            return ap
        d_siN = dout("d_siN", [8, 1024], F32)
        d_sjN = dout("d_sjN", [8, 1024], F32)
        d_cw = dout("d_cw", [128, 128 * Kout], F16)
        d_agg = dout("d_agg", [128, NG], F16)
        d_stats = dout("d_stats", [128, 2], F32)
        d_gstats = dout("d_gstats", [128, 2], F32)
        d_cf = dout("d_cf", [128, 6], F32)
        d_xT = dout("d_xT", [128, NG], F16)
        d_tT = dout("d_tT", [128, NG], F16)
        d_hT = dout("d_hT", [128, NG], F16)
        d_wt = dout("d_wt", [128, 8000], F16)

    cc_in = nc.dram_tensor("cc_in", [128, 2], F32).ap()
    cc_out = nc.dram_tensor("cc_out", [128, 2], F32, addr_space="Shared").ap()
    cc_win = nc.dram_tensor("cc_win", [128, 2], F32).ap()
    cc_wout = nc.dram_tensor("cc_wout", [128, 2], F32, addr_space="Shared").ap()

    # wpack column layout
    W_LIN, W_V, W_F2, W_F1, W_ATTQ, W_ATTEM, W_OUT, W_ID, W_ONE = (
        0, 128, 256, 384, 640, 642, 644, 645, 773)
    W_ONER = 774
    W_F2P = 902
    W_LINP = 1030
    # bpack columns: v_b, gnn_bias, f_b1, f_b2, bn_gamma, bn_beta
    B_VB, B_GNN, B_FB1, B_FB2, B_GAM, B_BET, B_EPS = 0, 1, 2, 3, 4, 5, 6

    with tile.TileContext(nc) as tc:
        with (
            tc.tile_pool(name="cst", bufs=1) as cst,
            tc.tile_pool(name="big", bufs=1) as big,
            tc.tile_pool(name="wt", bufs=3) as wtp,
            tc.tile_pool(name="sm", bufs=1) as sm,
            tc.tile_pool(name="stg", bufs=4) as stg,
            tc.tile_pool(name="wd", bufs=2) as wdp,
            tc.tile_pool(name="dnp", bufs=2) as dnp,
            tc.tile_pool(name="psA", bufs=4, space="PSUM") as psA,
            tc.tile_pool(name="psS", bufs=2, space="PSUM") as psS,
            tc.tile_pool(name="psT", bufs=2, space="PSUM") as psT,
        ):
            # ---- load constants (x0 first: biggest + on critical path)
            x0 = big.tile([128, NG], F16, tag="s16a")
            nc.sync.dma_start(x0[:], x0T)
            wp = cst.tile([128, 1184], F16)
            nc.sync.dma_start(wp[:], wpack)
            bp = cst.tile([128, 8], F32)
            nc.sync.dma_start(bp[:], bpack)
            ob = cst.tile([1, 1], F32)
            nc.sync.dma_start(ob[:], outb)
            emb = cst.tile([128, N], F16)
            nc.sync.dma_start(emb[:], embT)
            oidx = cst.tile([128, NOUT // 16], I16)
            nc.sync.dma_start(oidx[:], outidx_d)
            ocnt = big.tile([128, NOUT], F16, tag="cnt")
            nc.sync.dma_start(ocnt[:], outcnt_d)
            sidx = cst.tile([128, 8 * Kout], I16)
            nc.sync.dma_start(sidx[:], scatidx_d)

            def bias(col):
                return bp[:, col:col + 1]

            # warm up the collective path early (absorbs setup/skew)
            warm = sm.tile([128, 2], F32)
            nc.vector.memset(warm[:], 0.0)
            nc.sync.dma_start(cc_win, warm[:])
            nc.gpsimd.collective_compute(
                "AllReduce", ALU.add,
                replica_groups=[list(range(n_cores))],
                ins=[cc_win], outs=[cc_wout])

            # ---- B: xT = lin_w @ x0T  (scores depend on this -> first)
            xT = big.tile([128, NG], F16, tag="s16b")
            tT = big.tile([128, NG], F16, tag="tT")
            for h in range(NH):
                s = h * CH
                ps = psA.tile([128, CH], F32, tag="A")
                nc.tensor.matmul(ps[:], wp[:, W_LIN:W_LIN + 128],
                                 x0[:, s:s + CH], start=True, stop=True)
                nc.vector.tensor_copy(xT[:, s:s + CH], ps[:])

            # ---- D: node scores -> sNN [16, 1024]: rows 0-7 s_i, 8-15 s_j
            sNN = sm.tile([16, 1024], F32)
            nc.vector.memset(sNN[:], 0.0)
            emsc = sm.tile([2, N], F32)
            for h in range(2):
                ps = psS.tile([2, CH], F32, tag="S")
                nc.tensor.matmul(ps[:], wp[:, W_ATTEM:W_ATTEM + 2],
                                 emb[:, h * CH:(h + 1) * CH], start=True, stop=True)
                nc.vector.tensor_copy(emsc[:, h * CH:(h + 1) * CH], ps[:])
            attc = sm.tile([128, 2], F16)
            pat = psA.tile([128, CH], F32, tag="A")
            nc.tensor.matmul(pat[:, 0:2], wp[:, W_LINP:W_LINP + 128],
                             wp[:, W_ATTQ:W_ATTQ + 2], start=True, stop=True)
            nc.vector.tensor_copy(attc[:], pat[:, 0:2])
            for h in range(NH):
                s = h * CH
                g, off = divmod(s, 1000)
                ps = psS.tile([2, CH], F32, tag="S")
                nc.tensor.matmul(ps[:], attc[:, 0:2],
                                 x0[:, s:s + CH], start=True, stop=True)
                st = stg.tile([2, CH], F32, tag="sc")
                # add tiled emb scores while still [2, CH]
                nc.vector.tensor_tensor(st[:], ps[:], emsc[:, off:off + CH],
                                        op=ALU.add)
                nc.sync.dma_start(
                    sNN[:, off:off + CH].rearrange("(a g) f -> g a f", a=2)[g],
                    st[:, :])

            if dbg:
                nc.sync.dma_start(d_siN, sNN[0:8, :])
                nc.sync.dma_start(d_sjN, sNN[8:16, :])

            # ---- E: si table (for src gather) + chunk-local sj
            sjC = sm.tile([128, 128], F32)
            siT = sm.tile([128, 1024], F32)  # row 16c+g: s_i[g, :] (+0 sentinel)
            nc.vector.memset(siT[:], 0.0)
            for g in range(G):
                nc.sync.dma_start(sjC[g::16, :], sNN[8 + g:9 + g, :].rearrange(
                    "p (c f) -> p c f", c=8))
                bci = sNN[g:g + 1, 0:N].unsqueeze(1).broadcast_to([1, 8, N])
                nc.sync.dma_start(siT[g::16, 0:N], bci)
            NCH = 4
            NODC = 128 // NCH

            # ---- C: tT = v_w @ x0T + v_b (off critical path)
            for h in range(NH):
                s = h * CH
                ps2 = psA.tile([128, CH], F32, tag="A")
                nc.tensor.matmul(ps2[:], wp[:, W_V:W_V + 128],
                                 x0[:, s:s + CH], start=True, stop=True)
                nc.scalar.activation(tT[:, s:s + CH], ps2[:], AF.Identity,
                                     bias=bias(B_VB))

            # ---- G: src-phase -> unnormalized scatter values cw (chunked)
            g2 = big.tile([128, NOUT], F32, tag="gth")
            cw = big.tile([128, NOUT], F16, tag="cw")
            for q in range(NCH):
                sl = slice(q * NODC * Kout, (q + 1) * NODC * Kout)
                isl = slice(q * NODC * Kout // 16, (q + 1) * NODC * Kout // 16)
                nc.gpsimd.ap_gather(g2[:, sl], siT[:], oidx[:, isl], channels=128,
                                    num_elems=1024, d=1, num_idxs=NODC * Kout)
                nc.vector.tensor_tensor(
                    cw[:, sl].rearrange("p (n k) -> p n k", k=Kout),
                    g2[:, sl].rearrange("p (j k) -> p j k", k=Kout),
                    sjC[:, q * NODC:(q + 1) * NODC].unsqueeze(2).broadcast_to(
                        [128, NODC, Kout]),
                    op=ALU.add)
                nc.vector.scalar_tensor_tensor(cw[:, sl], cw[:, sl], NEG, cw[:, sl],
                                               op0=ALU.mult, op1=ALU.max)
                nc.scalar.activation(cw[:, sl], cw[:, sl], AF.Exp)
                nc.vector.tensor_tensor(cw[:, sl], cw[:, sl], ocnt[:, sl],
                                        op=ALU.mult)
            if dbg:
                nc.sync.dma_start(d_cw, cw[:])

            # ---- H: x_nm tiles (transpose xT per graph-tile) for agg lhsT
            xnm = big.tile([128, 64 * 128], F16, tag="s16a")
            for g in range(G):
                for t in range(8):
                    w = 128 if t < 7 else 104
                    pt = psT.tile([128, 128], F16, tag="T")
                    nc.tensor.transpose(pt[0:w, :],
                                        xT[:, g * 1000 + t * 128:
                                           g * 1000 + t * 128 + w],
                                        wp[:, W_ID:W_ID + 128])
                    nc.vector.tensor_copy(
                        xnm[0:w, (g * 8 + t) * 128:(g * 8 + t) * 128 + 128],
                        pt[0:w, :])

            # ---- I: per-graph W build + agg & den matmuls, normalize on evict
            aggT = big.tile([128, NG], F16, tag="agg")
            sqs = big.tile([128, NG], F16, tag="gth")
            sumacc = sm.tile([128, 8], F32)
            sqacc = sm.tile([128, 8], F32)
            for g in range(G):
                WT = wtp.tile([128, 8000], F16, tag="wt")
                wdata = wdp.tile([128, 8 * Kout], F16, tag="wd")
                for c in range(8):
                    nc.sync.dma_start(
                        wdata[:, c * Kout:(c + 1) * Kout],
                        cw[16 * c + g:16 * c + g + 1, :].rearrange(
                            "p (j k) -> p j k", k=Kout))
                for t in range(8):
                    nc.gpsimd.local_scatter(
                        WT[:, t * 1000:(t + 1) * 1000],
                        wdata[:, t * Kout:(t + 1) * Kout],
                        sidx[:, t * Kout:(t + 1) * Kout],
                        channels=128, num_elems=N, num_idxs=Kout)
                if dbg and g == G - 1:
                    nc.sync.dma_start(d_wt, WT[:])
                denR16 = dnp.tile([1, 1024], F16, tag="dn")
                for hf in range(2):
                    pd = psS.tile([2, CH], F32, tag="S")
                    for t in range(8):
                        kt = 128 if t < 7 else 104
                        nc.tensor.matmul(
                            pd[0:1, :], wp[0:kt, W_ONE:W_ONE + 1],
                            WT[0:kt, t * 1000 + hf * CH:t * 1000 + hf * CH + CH],
                            start=(t == 0), stop=(t == 7))
                    nc.vector.tensor_copy(denR16[0:1, hf * CH:hf * CH + CH],
                                          pd[0:1, :])
                # denFull = ones ⊗ den, then parallel reciprocal on 128 lanes
                rdf = wdp.tile([128, 1024], F32, tag="rdf")
                for hf in range(2):
                    pr = psA.tile([128, CH], F32, tag="A")
                    nc.tensor.matmul(pr[:], wp[0:1, W_ONER:W_ONER + 128],
                                     denR16[0:1, hf * CH:hf * CH + CH],
                                     start=True, stop=True)
                    nc.vector.reciprocal(rdf[:, hf * CH:hf * CH + CH], pr[:])
                for hf in range(2):
                    pa = psA.tile([128, CH], F32, tag="A")
                    for t in range(8):
                        kt = 128 if t < 7 else 104
                        nc.tensor.matmul(
                            pa[:], xnm[0:kt, (g * 8 + t) * 128:
                                       (g * 8 + t) * 128 + 128],
                            WT[0:kt, t * 1000 + hf * CH:t * 1000 + hf * CH + CH],
                            start=(t == 0), stop=(t == 7))
                    nc.vector.scalar_tensor_tensor(
                        aggT[:, g * 1000 + hf * CH:g * 1000 + hf * CH + CH],
                        pa[:], 1.0, rdf[:, hf * CH:hf * CH + CH],
                        op0=ALU.mult, op1=ALU.mult)
                # per-graph BN partial sums on ACT (idle during the loop)
                nc.scalar.activation(sqs[:, g * 1000:(g + 1) * 1000],
                                     aggT[:, g * 1000:(g + 1) * 1000],
                                     AF.Identity, accum_out=sumacc[:, g:g + 1])
                nc.scalar.activation(sqs[:, g * 1000:(g + 1) * 1000],
                                     aggT[:, g * 1000:(g + 1) * 1000],
                                     AF.Square, accum_out=sqacc[:, g:g + 1])
            # ---- J/K: BN stats (gnn_bias folded analytically) + AllReduce
            stats = sm.tile([128, 2], F32)
            s1u = sm.tile([128, 4], F32)
            nc.vector.tensor_reduce(s1u[:, 0:1], sumacc[:],
                                    axis=mybir.AxisListType.X, op=ALU.add)
            nc.vector.tensor_reduce(s1u[:, 1:2], sqacc[:],
                                    axis=mybir.AxisListType.X, op=ALU.add)
            # stats0 = sum(u*r) + Nn*gb ; stats1 = sum((u*r)^2) + 2*gb*sum(u*r) + Nn*gb^2
            gb = bias(B_GNN)
            nc.vector.tensor_scalar(s1u[:, 2:3], gb, float(B * N), None, op0=ALU.mult)
            nc.vector.tensor_tensor(stats[:, 0:1], s1u[:, 0:1], s1u[:, 2:3],
                                    op=ALU.add)
            nc.vector.tensor_tensor(s1u[:, 3:4], s1u[:, 2:3], gb, op=ALU.mult)
            # t = 2*s1 + Nn*gb ; stats1 = t*gb + sum_sq
            nc.vector.scalar_tensor_tensor(stats[:, 1:2], s1u[:, 0:1], 2.0,
                                           s1u[:, 2:3], op0=ALU.mult, op1=ALU.add)
            nc.vector.tensor_tensor(stats[:, 1:2], stats[:, 1:2], gb, op=ALU.mult)
            nc.vector.tensor_tensor(stats[:, 1:2], stats[:, 1:2], s1u[:, 1:2],
                                    op=ALU.add)
            if dbg:
                nc.sync.dma_start(d_stats, stats[:])
            nc.sync.dma_start(cc_in, stats[:])
            nc.gpsimd.collective_compute(
                "AllReduce", ALU.add,
                replica_groups=[list(range(n_cores))],
                ins=[cc_in], outs=[cc_out])
            gstats = sm.tile([128, 2], F32)
            nc.sync.dma_start(gstats[:], cc_out)
            if dbg:
                nc.sync.dma_start(d_gstats, gstats[:])

            # ---- L: BN coefficients A, Bv
            cf = sm.tile([128, 8], F32)
            mu, msq, var, rsd, A_, Bv = (cf[:, i:i + 1] for i in range(6))
            inv_n = 1.0 / (B * N)
            nc.vector.tensor_scalar_mul(mu, gstats[:, 0:1], inv_n)
            nc.vector.tensor_scalar_mul(msq, gstats[:, 1:2], inv_n)
            nc.vector.tensor_tensor(var, mu, mu, op=ALU.mult)
            nc.vector.tensor_sub(var, msq, var)
            nc.scalar.activation(var, var, AF.Sqrt, bias=bias(B_EPS))
            nc.vector.reciprocal(rsd, var)
            nc.vector.tensor_tensor(A_, bias(B_GAM), rsd, op=ALU.mult)
            nc.vector.tensor_tensor(Bv, mu, A_, op=ALU.mult)
            nc.vector.tensor_sub(Bv, bias(B_BET), Bv)
            nc.vector.tensor_tensor(cf[:, 6:7], bias(B_GNN), A_, op=ALU.mult)
            nc.vector.tensor_tensor(Bv, Bv, cf[:, 6:7], op=ALU.add)

            if dbg:
                nc.sync.dma_start(d_cf, cf[:])

            # ---- M: s_out = relu(A*agg + Bv) (in place; split ACT/DVE)
            HF2 = NG // 2
            nc.scalar.activation(aggT[:, 0:HF2], aggT[:, 0:HF2], AF.Relu,
                                 bias=Bv, scale=A_)
            nc.vector.tensor_scalar(aggT[:, HF2:NG], aggT[:, HF2:NG], A_, Bv,
                                    op0=ALU.mult, op1=ALU.add)
            nc.vector.tensor_scalar_max(aggT[:, HF2:NG], aggT[:, HF2:NG], 0.0)

            # ---- N/O/P: fusion MLP + out layer
            hT = big.tile([128, NG], F16, tag="s16a")
            for h in range(NH):
                s = h * CH
                ps = psA.tile([128, CH], F32, tag="A")
                nc.tensor.matmul(ps[:], wp[:, W_F1:W_F1 + 128],
                                 aggT[:, s:s + CH], start=True, stop=False)
                nc.tensor.matmul(ps[:], wp[:, W_F1 + 128:W_F1 + 256],
                                 tT[:, s:s + CH], start=False, stop=True)
                if h % 2 == 0:
                    nc.scalar.activation(hT[:, s:s + CH], ps[:], AF.Relu,
                                         bias=bias(B_FB1))
                else:
                    nc.vector.tensor_scalar(hT[:, s:s + CH], ps[:], bias(B_FB1),
                                            None, op0=ALU.add)
                    nc.vector.tensor_scalar_max(hT[:, s:s + CH], hT[:, s:s + CH],
                                                0.0)
            if dbg:
                nc.sync.dma_start(d_hT, hT[:])
            # composite head: c = f_w2 @ out_w ; cb = <out_w, f_b2> + out_b
            cvec = sm.tile([128, 2], F16)
            cb = sm.tile([1, 2], F32)
            nc.vector.tensor_copy(cvec[:, 1:2], bias(B_FB2))  # f_b2 -> f16
            pc = psS.tile([2, 1], F32, tag="S")
            nc.tensor.matmul(pc[0:1, 0:1], cvec[:, 1:2],
                             wp[:, W_OUT:W_OUT + 1], start=True, stop=True)
            pc2 = psA.tile([128, CH], F32, tag="A")
            nc.tensor.matmul(pc2[:, 0:1], wp[:, W_F2P:W_F2P + 128],
                             wp[:, W_OUT:W_OUT + 1], start=True, stop=True)
            nc.vector.tensor_copy(cvec[:, 0:1], pc2[:, 0:1])
            nc.vector.tensor_copy(cb[:, 0:1], pc[0:1, 0:1])
            nc.vector.tensor_tensor(cb[:, 1:2], cb[:, 0:1], ob[:], op=ALU.add)
            for h in range(NH):
                s = h * CH
                ps = psS.tile([2, CH], F32, tag="S")
                nc.tensor.matmul(ps[0:1, :], cvec[:, 0:1],
                                 hT[:, s:s + CH], start=True, stop=True)
                yst = stg.tile([2, CH], F32, tag="sc")
                nc.scalar.activation(yst[0:1, :], ps[0:1, :], AF.Identity,
                                     bias=cb[:, 1:2])
                nc.sync.dma_start(y_out[:, s:s + CH], yst[0:1, :])

    nc.compile()
    return nc


# ---------------------------------------------------------------- entry point
def _prepare(inputs):
    """Returns (nc, in_maps) — host prep + cached module build."""
    data = np.asarray(inputs["data"], np.float32)
    edge_index = np.asarray(inputs["edge_index"])

    pre = _prep_indices(edge_index)
    Kout = pre["Kout"]

    key = (Kout,)
    if key not in _CACHE:
        _CACHE[key] = _build(Kout, M)
    nc = _CACHE[key]

    f16 = np.float16

    def t16(a):  # transpose [r, c] -> [c, r] f16 contiguous
        return np.ascontiguousarray(np.asarray(a, np.float32).T).astype(f16)

    wpack = np.zeros((128, 1184), f16)
    wpack[:, 0:128] = t16(inputs["lin_w"])
    wpack[:, 128:256] = t16(inputs["v_w"])
    wpack[:, 256:384] = t16(inputs["f_w2"])
    wpack[:, 384:640] = np.ascontiguousarray(
        np.asarray(inputs["f_w1"], np.float32).T).astype(f16).reshape(2, 128, 128
        ).transpose(1, 0, 2).reshape(128, 256)
    wpack[:, 640] = np.asarray(inputs["att_i"], np.float32).astype(f16)
    wpack[:, 641] = np.asarray(inputs["att_j"], np.float32).astype(f16)
    wpack[:, 642] = np.asarray(inputs["att_em_i"], np.float32).astype(f16)
    wpack[:, 643] = np.asarray(inputs["att_em_j"], np.float32).astype(f16)
    wpack[:, 644] = np.asarray(inputs["out_w"], np.float32)[0].astype(f16)
    wpack[:, 645:773] = np.eye(128, dtype=f16)
    wpack[:, 773] = 1.0
    wpack[0, 774:902] = 1.0
    wpack[:, 902:1030] = np.asarray(inputs["f_w2"], np.float32).astype(f16)
    wpack[:, 1030:1158] = np.asarray(inputs["lin_w"], np.float32).astype(f16)

    bpack = np.zeros((128, 8), np.float32)
    bpack[:, 0] = np.asarray(inputs["v_b"], np.float32)
    bpack[:, 1] = np.asarray(inputs["gnn_bias"], np.float32)
    bpack[:, 2] = np.asarray(inputs["f_b1"], np.float32)
    bpack[:, 3] = np.asarray(inputs["f_b2"], np.float32)
    bpack[:, 4] = np.asarray(inputs["bn_gamma"], np.float32)
    bpack[:, 5] = np.asarray(inputs["bn_beta"], np.float32)
    bpack[:, 6] = EPS
    outb = np.asarray(inputs["out_b"], np.float32).reshape(1, 1)

    embT = t16(inputs["emb"])

    shared = dict(
        embT=embT, wpack=wpack, bpack=bpack, outb=outb,
        outidx=pre["outidx"], outcnt=pre["outcnt"], scatidx=pre["scatidx"],
    )
    in_maps = []
    for d in range(M):
        x0T = np.ascontiguousarray(
            data[d * G:(d + 1) * G].transpose(2, 0, 1).reshape(128, NG)
        ).astype(f16)
        in_maps.append(dict(shared, x0T=x0T))
    return nc, in_maps


def kernel(**inputs):
    nc, in_maps = _prepare(inputs)
    res = run_bass_kernel_spmd(nc, in_maps, list(range(M)))
    out = np.empty(B * N, np.float32)
    for d in range(M):
        out[d * NG:(d + 1) * NG] = res.results[d]["y"].reshape(-1)
    return out
